# revision 26
# baseline (speedup 1.0000x reference)
"""AttentionClustering (vq_codebook) Trainium2 kernel, 8-core data parallel.

Shard: 8 cores = 4 images x 2 half-images (128 output rows each). Odd cores
get a vertically flipped shard + row-flipped conv weights so every core's
program is identical (true image edge at local top, interior halo at bottom).

Math: q1 = relu(conv3x3(x, w1) + b1); q2 = relu(conv3x3(q1, w2) + b2)  (both
with replicate padding); then the 1x1 conv + cluster-distance softmax folds to
  logit[px, k] = sum_ci q2[ci, px] * muW[k, ci] + bp[k]
  muW = 2 * mu @ W3,  bp = 2 * mu @ b3 - |mu|^2      (|q|^2 cancels in softmax)
  out[px] = sum_k softmax_k(logit) * label[k]

conv2 (66% of the FLOPs) runs as 1D Winograd F(2,3) along image columns with
the 3 row-taps kept direct: per 2-col output tile, 4 transformed points j with
  t0 = d0-d2, t1 = d1+d2, t2 = d1-d2, t3 = d1-d3   (DVE, all plain adds)
  M_j = sum_dr U[j,dr] @ V_j(rows r+dr)            (PE, N=512 matmuls)
  y0 = M0+M1+M2, y1 = M1-M2-M3                     (DVE + one ACT copy)
cutting conv2 PE cycles 18/12 = 1.5x. q1 is stored column-de-interleaved
(odd/even planes) so the winograd d-reads are contiguous; conv1's matmuls
write PSUM already de-interleaved via a rearranged moving AP. conv1 keeps
direct matmuls from a single (ch, row-pair) packed x buffer; its 6 K=64
dr=2-taps run pairwise concurrent via tile_position row groups. Work is
emitted in 4-row strip-pairs, mc-phased over 4 PSUM banks; conv1 +
in-transform of band b+1 are interleaved into band b's pairs so PE never
waits on DVE. All matmuls fp16 (measured end-to-end rel err 1.8e-3).
"""
import os
import sys
if '/opt/trn_rl_repo' not in sys.path:
    sys.path.insert(0, '/opt/trn_rl_repo')

import numpy as np
import concourse.bass as bass
import concourse.mybir as mybir
from concourse import bacc, tile
from concourse.bass_utils import run_bass_kernel_spmd

F32 = mybir.dt.float32
F16 = mybir.dt.float16
AF = mybir.ActivationFunctionType
ALU = mybir.AluOpType
AX = mybir.AxisListType

B, CIN, H, W = 4, 64, 256, 256
Q, K = 256, 16
RB = 32           # output rows per band
NBAND = 4         # bands per core (128 rows)
NCORES = 8
TC = W // 2       # winograd 2-col tiles per row
NPAIR = RB // 4   # 4-row strip-pairs per band
XROWS = RB + 4    # x halo rows per band buffer
WARM = 26

_cached = {}


def build_nc():
    nc = bacc.Bacc("TRN2", target_bir_lowering=False, debug=False)

    CHS = 133 * (W + 2)          # per-channel element stride in flat xh
    xh = nc.declare_dram_parameter("xh", [CIN * CHS], F16, isOutput=False)
    w1d = nc.declare_dram_parameter("w1d", [128, 6, 128], F16, isOutput=False)
    w1e = nc.declare_dram_parameter("w1e", [128, 3, 128], F16, isOutput=False)
    w2u = nc.declare_dram_parameter("w2u", [128, 48, 128], F16, isOutput=False)
    muw = nc.declare_dram_parameter("muw", [128, 2, K], F16, isOutput=False)
    cst = nc.declare_dram_parameter("cst", [128, 2 * K + 4], F32, isOutput=False)
    outd = nc.declare_dram_parameter("out", [128, NBAND * NPAIR * 8], F32,
                                     isOutput=True)

    with tile.TileContext(nc) as tc:
        with tc.tile_pool(name="singles", bufs=1) as singles, \
             tc.tile_pool(name="xpool", bufs=2) as xpool, \
             tc.tile_pool(name="q1pool", bufs=1) as q1pool, \
             tc.tile_pool(name="vpool", bufs=2) as vpool, \
             tc.tile_pool(name="ypool", bufs=2) as ypool, \
             tc.tile_pool(name="q2pool", bufs=2) as q2pool, \
             tc.tile_pool(name="wtp", bufs=2) as wtp, \
             tc.tile_pool(name="smx", bufs=2) as smx, \
             tc.tile_pool(name="obuf", bufs=2) as obuf, \
             tc.tile_pool(name="c1ps", bufs=1, space="PSUM") as c1ps, \
             tc.tile_pool(name="wps", bufs=4, space="PSUM") as wps, \
             tc.tile_pool(name="psl", bufs=2, space="PSUM") as psl:

            # ---- resident weights (one DMA per family) ----------------
            w1dbuf = singles.tile([128, 6, 128], F16, tag="w1dbuf")
            nc.sync.dma_start(out=w1dbuf, in_=w1d.ap())
            w1d_sb = {(dc, mc): w1dbuf[:, dc * 2 + mc, :]
                      for dc in range(3) for mc in range(2)}
            w1ebuf = singles.tile([128, 3, 128], F16, tag="w1ebuf")
            nc.sync.dma_start(out=w1ebuf, in_=w1e.ap())

            xh_ap = xh.ap()

            def xsrc(r0, lo, hi, shift):
                return bass.AP(
                    tensor=xh_ap.tensor,
                    offset=(r0 + lo) * (W + 2) + shift,
                    ap=[[CHS, CIN], [W + 2, hi - lo], [1, W + 2]])

            def load_xband(r0, chunks):
                # partitions 0-63: x halo rows r0+jj; 64-127: rows r0+jj+1
                xr = xpool.tile([128, XROWS, W + 2], F16, tag="xr", name="xr")
                for lo, hi in chunks:
                    nc.sync.dma_start(out=xr[0:64, lo:hi, :],
                                      in_=xsrc(r0, lo, hi, 0))
                    nc.sync.dma_start(out=xr[64:128, lo:hi, :],
                                      in_=xsrc(r0, lo, hi, W + 2))
                return xr

            xr_bufs = {0: load_xband(0, [(0, 10), (10, 20), (20, 28), (28, 36)])}

            # small constants next (before the bulky w2 tiles hog the queues)
            muwbuf = singles.tile([128, 2, K], F16, tag="muwbuf")
            nc.sync.dma_start(out=muwbuf, in_=muw.ap())
            muw_sb = {kc: muwbuf[:, kc, :] for kc in range(2)}
            cstbuf = singles.tile([128, 2 * K + 4], F32, tag="cstbuf")
            nc.sync.dma_start(out=cstbuf, in_=cst.ap())
            bp_sb = cstbuf[:, 0:K]
            lab_sb = cstbuf[:, K:2 * K]
            b1_sb = {mc: cstbuf[:, 2 * K + mc:2 * K + mc + 1] for mc in range(2)}
            b2_sb = {mc: cstbuf[:, 2 * K + 2 + mc:2 * K + 3 + mc] for mc in range(2)}

            # PE warmup: keep TensorE busy through the initial DMA wait so
            # the HAM clock-gate is at 8/8 when real matmuls arrive.
            wscr = singles.tile([128, 512], F16, tag="wscr")
            nc.vector.memset(wscr, 0.0)
            wu = wps.tile([128, 4, TC], F32, tag="wps", name="wu")
            for _ in range(WARM):
                nc.tensor.matmul(wu, wscr[:, 0:128], wscr,
                                 start=True, stop=True)

            w2ubuf = singles.tile([128, 48, 128], F16, tag="w2ubuf")
            nc.sync.dma_start(out=w2ubuf, in_=w2u.ap())
            w2u_sb = {(j, dr, kc, mc): w2ubuf[:, ((j * 3 + dr) * 2 + kc) * 2 + mc, :]
                      for j in range(4) for dr in range(3)
                      for kc in range(2) for mc in range(2)}

            # q1 stored de-interleaved: dim2=0 -> odd padded cols (1,3,..,257),
            # dim2=1 -> even padded cols (0,2,..,256); winograd d-reads are
            # then contiguous 128-col slices.
            q1b = {kc: q1pool.tile([128, RB + 2, 2, TC + 1], F16,
                                   tag=f"q1_{kc}", name=f"q1_{kc}")
                   for kc in range(2)}

            def q1_act_out(kc, j, nr):
                # psum is [nr, 2, TC] (even cols then odd cols per row):
                # even out col 2i -> odd-plane[i], odd col 2i+1 -> even[i+1]
                base = q1b[kc][:, 0, 0, 0:1]
                rowsz = 2 * (TC + 1)
                return bass.AP(
                    tensor=base.tensor, offset=base.offset + j * rowsz,
                    ap=[base.ap[0], [rowsz, nr], [TC + 2, 2], [1, TC]])

            # ---- emission helpers --------------------------------------
            def conv1_group(xr, j, nr):
                # q1 slots j..j+nr-1 (both mc halves) from x halo rows
                ct = {0: c1ps.tile([128, nr, 2, TC], F32, tag="c1a", name="c1a"),
                      1: c1ps.tile([128, nr, 2, TC], F32, tag="c1b", name="c1b")}
                for mc in range(2):
                    for dc in range(3):
                        mov = xr[:, j:j + nr, dc:dc + W].rearrange(
                            "p r (i h) -> p r h i", h=2)
                        nc.tensor.matmul(ct[mc], w1d_sb[dc, mc], mov,
                                         start=(dc == 0), stop=False)
                # dr=2 taps: K=64 pairs run concurrently on row-groups 0/64
                for dc in range(3):
                    nc.tensor.matmul(
                        ct[0], w1ebuf[0:64, dc, :],
                        xr[0:64, j + 2:j + 2 + nr, dc:dc + W].rearrange(
                            "p r (i h) -> p r h i", h=2),
                        start=False, stop=(dc == 2))
                    nc.tensor.matmul(
                        ct[1], w1ebuf[64:128, dc, :],
                        xr[64:128, j + 1:j + 1 + nr, dc:dc + W].rearrange(
                            "p r (i h) -> p r h i", h=2),
                        start=False, stop=(dc == 2),
                        tile_position=(64, 0))
                for mc in range(2):
                    nc.scalar.activation(
                        out=q1_act_out(mc, j, nr), in_=ct[mc],
                        func=AF.Relu, bias=b1_sb[mc], scale=1.0)

            def emit_pads(band):
                lo = 1 if band == 0 else 0
                for kc in range(2):
                    # col 0 (even[0]) <- col 1 (odd[0]); col 257 (odd[128])
                    # <- col 256 (even[128])
                    nc.vector.tensor_copy(
                        out=q1b[kc][:, lo:RB + 2, 1, 0:1],
                        in_=q1b[kc][:, lo:RB + 2, 0, 0:1])
                    nc.vector.tensor_copy(
                        out=q1b[kc][:, lo:RB + 2, 0, TC:TC + 1],
                        in_=q1b[kc][:, lo:RB + 2, 1, TC:TC + 1])
                    if band == 0:
                        nc.vector.tensor_copy(
                            out=q1b[kc][:, 0:1, :, :], in_=q1b[kc][:, 1:2, :, :])

            # in-transform ops for one (half, kc, j): 16 per band.
            # d0=even[tc] (plane1), d1=odd[tc] (plane0), d2=even[tc+1],
            # d3=odd[tc+1]: t0=d0-d2, t1=d1+d2, t2=d1-d2, t3=d1-d3.
            IT_OPS = [(0, (1, 0), (1, 1), ALU.subtract),
                      (1, (0, 0), (1, 1), ALU.add),
                      (2, (0, 0), (1, 1), ALU.subtract),
                      (3, (0, 0), (0, 1), ALU.subtract)]

            def intf_op(vh, h, kc, oi, eng=None):
                j, (pa, oa), (pb, ob_), op = IT_OPS[oi]
                rr = slice(16 * h, 16 * h + 18)
                (eng or nc.gpsimd).tensor_tensor(
                    vh[:, kc, j, :, :],
                    q1b[kc][:, rr, pa, oa:oa + TC],
                    q1b[kc][:, rr, pb, ob_:ob_ + TC], op)

            def new_vh():
                return vpool.tile([128, 2, 4, 18, TC], F16, tag="vh", name="vh")

            GROUPS0 = [(j, 2) for j in range(1, RB + 1, 2)] + [(RB + 1, 1)]
            GROUPS = [(j, 2) for j in range(0, RB + 2, 2)]

            CH2 = [(0, 18), (18, 36)]

            # ---- band 0 prologue ---------------------------------------
            for j, nr in GROUPS0:
                conv1_group(xr_bufs[0], j, nr)
            emit_pads(0)
            vh_cur = new_vh()
            for kc in range(2):
                for oi in range(4):
                    # DVE here: the band-0 head has no other DVE work
                    intf_op(vh_cur, 0, kc, oi, eng=nc.vector)
            xr_bufs[1] = load_xband(RB, CH2)

            # ---- bands: 8 strip-pairs of 4 output rows each -------------
            for band in range(NBAND):
                ob = obuf.tile([128, NPAIR, 8], F32, tag="ob", name="ob")
                nb = band + 1
                for gp in range(NPAIR):
                    h = gp // 4
                    if gp == 4:
                        vh_cur = vh_pending
                    lr0 = 4 * gp - 16 * h
                    q2t = q2pool.tile([128, 2, 4, 2, TC], F16, tag="q2t",
                                      name="q2t")
                    for mc in range(2):
                        ps = [wps.tile([128, 4, TC], F32, tag="wps",
                                       name=f"wps{j}") for j in range(4)]
                        for j in range(4):
                            n = 0
                            for kc in range(2):
                                for dr in range(3):
                                    nc.tensor.matmul(
                                        ps[j], w2u_sb[j, dr, kc, mc],
                                        vh_cur[:, kc, j, lr0 + dr:lr0 + dr + 4, :],
                                        start=(n == 0), stop=(n == 5))
                                    n += 1
                        # out-transform: y0 = m0+m1+m2, y1 = m1-m2-m3
                        yt = ypool.tile([128, 4, 2, TC], F32, tag="yt", name="yt")
                        wa = wtp.tile([128, 4, TC], F32, tag="wa", name="wa")
                        wb = wtp.tile([128, 4, TC], F32, tag="wb", name="wb")
                        s1 = wtp.tile([128, 4, TC], F32, tag="s1", name="s1")
                        nc.scalar.activation(out=s1, in_=ps[1], func=AF.Copy)
                        nc.vector.tensor_tensor(wa, ps[0], s1, ALU.add)
                        nc.vector.tensor_tensor(yt[:, :, 0, :], wa, ps[2], ALU.add)
                        nc.vector.tensor_tensor(wb, s1, ps[2], ALU.subtract)
                        nc.vector.tensor_tensor(yt[:, :, 1, :], wb, ps[3],
                                                ALU.subtract)
                        nc.scalar.activation(out=q2t[:, mc], in_=yt,
                                             func=AF.Relu, bias=b2_sb[mc],
                                             scale=1.0)
                    # logits: [128 tc-px, K] per (r, s) chunk, q2 stationary
                    pl = psl.tile([128, 8, K], F32, tag="lps", name="lps")
                    for cc in range(8):
                        r, s = cc // 2, cc % 2
                        for kc in range(2):
                            nc.tensor.matmul(
                                pl[:, cc, :], q2t[:, kc, r, s, :],
                                muw_sb[kc], start=(kc == 0), stop=(kc == 1))
                    # softmax over K (free axis) + label contraction
                    li = smx.tile([128, 8, K], F32, tag="li", name="li")
                    nc.vector.tensor_tensor(
                        li, pl,
                        bp_sb.unsqueeze(1).to_broadcast([128, 8, K]), ALU.add)
                    mx = smx.tile([128, 8], F32, tag="mx", name="mx")
                    nc.vector.reduce_max(mx, li, axis=AX.X)
                    ls = smx.tile([128, 8, K], F32, tag="ls", name="ls")
                    nc.vector.tensor_tensor(
                        ls, li,
                        mx.unsqueeze(2).to_broadcast([128, 8, K]), ALU.subtract)
                    ex = smx.tile([128, 8, K], F32, tag="ex", name="ex")
                    nc.scalar.activation(out=ex, in_=ls, func=AF.Exp)
                    el = smx.tile([128, 8, K], F32, tag="el", name="el")
                    nc.vector.tensor_tensor(
                        el, ex,
                        lab_sb.unsqueeze(1).to_broadcast([128, 8, K]), ALU.mult)
                    ssum = smx.tile([128, 8], F32, tag="ssum", name="ssum")
                    nc.vector.reduce_sum(ssum, ex, axis=AX.X)
                    wsum = smx.tile([128, 8], F32, tag="wsum", name="wsum")
                    nc.vector.reduce_sum(wsum, el, axis=AX.X)
                    rs = smx.tile([128, 8], F32, tag="rs", name="rs")
                    nc.vector.reciprocal(rs, ssum)
                    nc.vector.tensor_tensor(ob[:, gp], wsum, rs, ALU.mult)

                    # ---- cross-band interleaves ------------------------
                    if gp < 2:
                        # second-half V of current band
                        if gp == 0:
                            vh_pending = new_vh()
                        for o in range(4 * gp, 4 * gp + 4):
                            intf_op(vh_pending, 1, o // 4, o % 4)
                    if nb < NBAND:
                        if gp in (1, 2) and nb >= 2:
                            if gp == 1:
                                xr_bufs[nb] = load_xband(RB * nb, CH2[:1])
                            else:
                                lo, hi = CH2[1]
                                nc.sync.dma_start(
                                    out=xr_bufs[nb][0:64, lo:hi, :],
                                    in_=xsrc(RB * nb, lo, hi, 0))
                                nc.sync.dma_start(
                                    out=xr_bufs[nb][64:128, lo:hi, :],
                                    in_=xsrc(RB * nb, lo, hi, W + 2))
                        if 2 <= gp <= 5:
                            i0 = (gp - 2) * 17 // 4
                            i1 = (gp - 1) * 17 // 4
                            for j, nr in GROUPS[i0:i1]:
                                conv1_group(xr_bufs[nb], j, nr)
                            if gp == 5:
                                emit_pads(nb)
                        if gp >= 6:
                            if gp == 6:
                                vh_nx = new_vh()
                            for o in range(4 * (gp - 6), 4 * (gp - 6) + 4):
                                intf_op(vh_nx, 0, o // 4, o % 4)

                nc.sync.dma_start(
                    out=outd.ap()[:, band * NPAIR * 8:(band + 1) * NPAIR * 8],
                    in_=ob)
                if nb < NBAND:
                    vh_cur = vh_nx

    nc.compile()
    return nc


def prep_inputs(x, w1, b1, w2, b2, w3, b3, mu, label):
    """Full inputs -> per-core in_maps."""
    w3m = w3[:, :, 0, 0]
    muW = 2.0 * (mu @ w3m)                                   # [K, Q]
    bpv = (2.0 * (mu @ b3) - (mu * mu).sum(1)).astype(np.float32)

    def pack_w(w1f, w2f):
        w1dp = np.empty((128, 6, 128), np.float32)
        for u in range(2):
            for dc in range(3):
                for mc in range(2):
                    w1dp[u * 64:(u + 1) * 64, dc * 2 + mc, :] = \
                        w1f[mc * 128:(mc + 1) * 128, :, u, dc].T
        w1ep = np.empty((128, 3, 128), np.float32)
        for dc in range(3):
            w1ep[0:64, dc, :] = w1f[0:128, :, 2, dc].T
            w1ep[64:128, dc, :] = w1f[128:256, :, 2, dc].T
        U = np.empty((4, 3, Q, Q), np.float32)
        for dr in range(3):
            g0, g1, g2 = (w2f[:, :, dr, dc] for dc in range(3))
            U[0, dr] = g0
            U[1, dr] = (g0 + g1 + g2) * 0.5
            U[2, dr] = (g1 - g0 - g2) * 0.5   # sign-folded for t2 = d1-d2
            U[3, dr] = g2
        w2up = np.empty((128, 48, 128), np.float32)
        for j in range(4):
            for dr in range(3):
                for kc in range(2):
                    for mc in range(2):
                        idx = ((j * 3 + dr) * 2 + kc) * 2 + mc
                        w2up[:, idx, :] = U[j, dr][
                            mc * 128:(mc + 1) * 128,
                            kc * 128:(kc + 1) * 128].T
        return (w1dp.astype(np.float16), w1ep.astype(np.float16),
                w2up.astype(np.float16))

    packs = {}
    packs[0] = pack_w(w1, w2)
    packs[1] = pack_w(w1[:, :, ::-1, :], w2[:, :, ::-1, :])

    muwp = np.empty((128, 2, K), np.float32)
    for kc in range(2):
        muwp[:, kc, :] = muW[:, 128 * kc:128 * (kc + 1)].T
    muwp = muwp.astype(np.float16)
    cstv = np.empty((128, 2 * K + 4), np.float32)
    cstv[:, 0:K] = bpv[None, :]
    cstv[:, K:2 * K] = label[None, :].astype(np.float32)
    for mc in range(2):
        cstv[:, 2 * K + mc] = b1[128 * mc:128 * (mc + 1)]
        cstv[:, 2 * K + 2 + mc] = b2[128 * mc:128 * (mc + 1)]

    rows = np.clip(np.arange(133) - 2, 0, H - 1)
    cols = np.clip(np.arange(W + 2) - 1, 0, W - 1)
    in_maps = []
    for core in range(NCORES):
        img, half = core // 2, core % 2
        xl = x[img] if half == 0 else x[img, :, ::-1, :]
        xhv = np.ascontiguousarray(xl[:, rows][:, :, cols]).astype(np.float16)
        w1dp, w1ep, w2up = packs[half]
        in_maps.append({
            'xh': xhv.reshape(-1), 'w1d': w1dp, 'w1e': w1ep,
            'w2u': w2up, 'muw': muwp, 'cst': cstv,
        })
    return in_maps


def gather(results, dtype=np.float32):
    out = np.empty((B, 1, H, W), dtype)
    for core in range(NCORES):
        img, half = core // 2, core % 2
        o = results[core]['out'].reshape(128, NBAND, NPAIR, 4, 2)
        # [p, band, gp, r, s]: row = 32*band + 4*gp + r, col = 2p + s
        o = o.transpose(1, 2, 3, 0, 4).reshape(128, W)
        if half == 0:
            out[img, 0, 0:128] = o
        else:
            out[img, 0, 128:256] = o[::-1]
    return out


def get_nc():
    if 'nc' not in _cached:
        _cached['nc'] = build_nc()
    return _cached['nc']


def kernel(x, w1, b1, w2, b2, w3, b3, mu, label, **run_kwargs):
    nc = get_nc()
    in_maps = prep_inputs(
        np.asarray(x, np.float32), np.asarray(w1, np.float32),
        np.asarray(b1, np.float32), np.asarray(w2, np.float32),
        np.asarray(b2, np.float32), np.asarray(w3, np.float32),
        np.asarray(b3, np.float32), np.asarray(mu, np.float32),
        np.asarray(label, np.float32))
    res = run_bass_kernel_spmd(nc, in_maps, core_ids=list(range(NCORES)),
                               **run_kwargs)
    out = gather(res.results)
    if run_kwargs:
        _cached['last_result'] = res
    return out


# revision 27
# speedup vs baseline: 1.1614x; 1.1614x over previous
"""AttentionClustering (vq_codebook) Trainium2 kernel, 8-core data parallel.

Shard: 8 cores = 4 images x 2 half-images (128 output rows each). Odd cores
get a vertically flipped shard + row-flipped conv weights so every core's
program is identical (true image edge at local top, interior halo at bottom).

Math: q1 = relu(conv3x3(x, w1) + b1); q2 = relu(conv3x3(q1, w2) + b2)  (both
with replicate padding); then the 1x1 conv + cluster-distance softmax folds to
  logit[px, k] = sum_ci q2[ci, px] * muW[k, ci] + bp[k]
  muW = 2 * mu @ W3,  bp = 2 * mu @ b3 - |mu|^2      (|q|^2 cancels in softmax)
  out[px] = sum_k softmax_k(logit) * label[k]

conv2 (66% of the FLOPs) runs as 1D Winograd F(2,3) along image columns with
the 3 row-taps kept direct: per 2-col output tile, 4 transformed points j with
  t0 = d0-d2, t1 = d1+d2, t2 = d1-d2, t3 = d1-d3   (DVE, all plain adds)
  M_j = sum_dr U[j,dr] @ V_j(rows r+dr)            (PE, N=512 matmuls)
  y0 = M0+M1+M2, y1 = M1-M2-M3                     (DVE + one ACT copy)
cutting conv2 PE cycles 18/12 = 1.5x. q1 is stored column-de-interleaved
(odd/even planes) so the winograd d-reads are contiguous; conv1's matmuls
write PSUM already de-interleaved via a rearranged moving AP. conv1 keeps
direct matmuls from a single (ch, row-pair) packed x buffer; its 6 K=64
dr=2-taps run pairwise concurrent via tile_position row groups. Work is
emitted in 4-row strip-pairs, mc-phased over 4 PSUM banks; conv1 +
in-transform of band b+1 are interleaved into band b's pairs so PE never
waits on DVE. All matmuls fp16 (measured end-to-end rel err 1.8e-3).
"""
import os
import sys
if '/opt/trn_rl_repo' not in sys.path:
    sys.path.insert(0, '/opt/trn_rl_repo')

import numpy as np
import concourse.bass as bass
import concourse.mybir as mybir
from concourse import bacc, tile
from concourse.bass_utils import run_bass_kernel_spmd

F32 = mybir.dt.float32
F16 = mybir.dt.float16
AF = mybir.ActivationFunctionType
ALU = mybir.AluOpType
AX = mybir.AxisListType

B, CIN, H, W = 4, 64, 256, 256
Q, K = 256, 16
RB = 32           # output rows per band
NBAND = 4         # bands per core (128 rows)
NCORES = 8
TC = W // 2       # winograd 2-col tiles per row
NPAIR = RB // 4   # 4-row strip-pairs per band
XROWS = RB + 4    # x halo rows per band buffer
WARM = 26

_cached = {}


def build_nc():
    nc = bacc.Bacc("TRN2", target_bir_lowering=False, debug=False)

    CHS = 133 * (W + 2)          # per-channel element stride in flat xh
    xh = nc.declare_dram_parameter("xh", [CIN * CHS], F16, isOutput=False)
    w1d = nc.declare_dram_parameter("w1d", [128, 6, 128], F16, isOutput=False)
    w1e = nc.declare_dram_parameter("w1e", [128, 3, 128], F16, isOutput=False)
    w2u = nc.declare_dram_parameter("w2u", [128, 48, 128], F16, isOutput=False)
    muw = nc.declare_dram_parameter("muw", [128, 2, K], F16, isOutput=False)
    cst = nc.declare_dram_parameter("cst", [128, 2 * K + 4], F32, isOutput=False)
    outd = nc.declare_dram_parameter("out", [128, NBAND * NPAIR * 8], F32,
                                     isOutput=True)

    with tile.TileContext(nc) as tc:
        with tc.tile_pool(name="singles", bufs=1) as singles, \
             tc.tile_pool(name="xpool", bufs=2) as xpool, \
             tc.tile_pool(name="q1pool", bufs=1) as q1pool, \
             tc.tile_pool(name="vpool", bufs=2) as vpool, \
             tc.tile_pool(name="ypool", bufs=2) as ypool, \
             tc.tile_pool(name="q2pool", bufs=2) as q2pool, \
             tc.tile_pool(name="wtp", bufs=2) as wtp, \
             tc.tile_pool(name="smx", bufs=2) as smx, \
             tc.tile_pool(name="obuf", bufs=2) as obuf, \
             tc.tile_pool(name="c1ps", bufs=1, space="PSUM") as c1ps, \
             tc.tile_pool(name="wps", bufs=4, space="PSUM") as wps, \
             tc.tile_pool(name="psl", bufs=2, space="PSUM") as psl:

            # ---- resident weights (one DMA per family) ----------------
            w1dbuf = singles.tile([128, 6, 128], F16, tag="w1dbuf")
            nc.sync.dma_start(out=w1dbuf, in_=w1d.ap())
            w1d_sb = {(dc, mc): w1dbuf[:, dc * 2 + mc, :]
                      for dc in range(3) for mc in range(2)}
            w1ebuf = singles.tile([128, 3, 128], F16, tag="w1ebuf")
            nc.sync.dma_start(out=w1ebuf, in_=w1e.ap())

            xh_ap = xh.ap()

            def xsrc(r0, lo, hi, shift):
                return bass.AP(
                    tensor=xh_ap.tensor,
                    offset=(r0 + lo) * (W + 2) + shift,
                    ap=[[CHS, CIN], [W + 2, hi - lo], [1, W + 2]])

            def load_xband(r0, chunks):
                # partitions 0-63: x halo rows r0+jj; 64-127: rows r0+jj+1
                xr = xpool.tile([128, XROWS, W + 2], F16, tag="xr", name="xr")
                for lo, hi in chunks:
                    nc.sync.dma_start(out=xr[0:64, lo:hi, :],
                                      in_=xsrc(r0, lo, hi, 0))
                    nc.sync.dma_start(out=xr[64:128, lo:hi, :],
                                      in_=xsrc(r0, lo, hi, W + 2))
                return xr

            xr_bufs = {0: load_xband(0, [(0, 10), (10, 20), (20, 28), (28, 36)])}

            # small constants next (before the bulky w2 tiles hog the queues)
            muwbuf = singles.tile([128, 2, K], F16, tag="muwbuf")
            nc.sync.dma_start(out=muwbuf, in_=muw.ap())
            muw_sb = {kc: muwbuf[:, kc, :] for kc in range(2)}
            cstbuf = singles.tile([128, 2 * K + 4], F32, tag="cstbuf")
            nc.sync.dma_start(out=cstbuf, in_=cst.ap())
            bp_sb = cstbuf[:, 0:K]
            lab_sb = cstbuf[:, K:2 * K]
            b1_sb = {mc: cstbuf[:, 2 * K + mc:2 * K + mc + 1] for mc in range(2)}
            b2_sb = {mc: cstbuf[:, 2 * K + 2 + mc:2 * K + 3 + mc] for mc in range(2)}

            # PE warmup: keep TensorE busy through the initial DMA wait so
            # the HAM clock-gate is at 8/8 when real matmuls arrive.
            wscr = singles.tile([128, 512], F16, tag="wscr")
            nc.vector.memset(wscr, 0.0)
            wu = wps.tile([128, 4, TC], F32, tag="wps", name="wu")
            for _ in range(WARM):
                nc.tensor.matmul(wu, wscr[:, 0:128], wscr,
                                 start=True, stop=True)

            w2ubuf = singles.tile([128, 48, 128], F16, tag="w2ubuf")
            nc.sync.dma_start(out=w2ubuf, in_=w2u.ap())
            w2u_sb = {(j, dr, kc, mc): w2ubuf[:, ((j * 3 + dr) * 2 + kc) * 2 + mc, :]
                      for j in range(4) for dr in range(3)
                      for kc in range(2) for mc in range(2)}

            # q1 stored de-interleaved: dim2=0 -> odd padded cols (1,3,..,257),
            # dim2=1 -> even padded cols (0,2,..,256); winograd d-reads are
            # then contiguous 128-col slices.
            q1b = {kc: q1pool.tile([128, RB + 2, 2, TC + 1], F16,
                                   tag=f"q1_{kc}", name=f"q1_{kc}")
                   for kc in range(2)}

            def q1_act_out(kc, j, nr):
                # psum is [nr, 2, TC] (even cols then odd cols per row):
                # even out col 2i -> odd-plane[i], odd col 2i+1 -> even[i+1]
                base = q1b[kc][:, 0, 0, 0:1]
                rowsz = 2 * (TC + 1)
                return bass.AP(
                    tensor=base.tensor, offset=base.offset + j * rowsz,
                    ap=[base.ap[0], [rowsz, nr], [TC + 2, 2], [1, TC]])

            # ---- emission helpers --------------------------------------
            def conv1_group(xr, j, nr):
                # q1 slots j..j+nr-1 (both mc halves) from x halo rows
                ct = {0: c1ps.tile([128, nr, 2, TC], F32, tag="c1a", name="c1a"),
                      1: c1ps.tile([128, nr, 2, TC], F32, tag="c1b", name="c1b")}
                for mc in range(2):
                    for dc in range(3):
                        mov = xr[:, j:j + nr, dc:dc + W].rearrange(
                            "p r (i h) -> p r h i", h=2)
                        nc.tensor.matmul(ct[mc], w1d_sb[dc, mc], mov,
                                         start=(dc == 0), stop=False)
                # dr=2 taps: K=64 pairs run concurrently on row-groups 0/64
                for dc in range(3):
                    nc.tensor.matmul(
                        ct[0], w1ebuf[0:64, dc, :],
                        xr[0:64, j + 2:j + 2 + nr, dc:dc + W].rearrange(
                            "p r (i h) -> p r h i", h=2),
                        start=False, stop=(dc == 2))
                    nc.tensor.matmul(
                        ct[1], w1ebuf[64:128, dc, :],
                        xr[64:128, j + 1:j + 1 + nr, dc:dc + W].rearrange(
                            "p r (i h) -> p r h i", h=2),
                        start=False, stop=(dc == 2),
                        tile_position=(64, 0))
                for mc in range(2):
                    nc.scalar.activation(
                        out=q1_act_out(mc, j, nr), in_=ct[mc],
                        func=AF.Relu, bias=b1_sb[mc], scale=1.0)

            def emit_pads(band):
                lo = 1 if band == 0 else 0
                for kc in range(2):
                    # col 0 (even[0]) <- col 1 (odd[0]); col 257 (odd[128])
                    # <- col 256 (even[128])
                    nc.vector.tensor_copy(
                        out=q1b[kc][:, lo:RB + 2, 1, 0:1],
                        in_=q1b[kc][:, lo:RB + 2, 0, 0:1])
                    nc.vector.tensor_copy(
                        out=q1b[kc][:, lo:RB + 2, 0, TC:TC + 1],
                        in_=q1b[kc][:, lo:RB + 2, 1, TC:TC + 1])
                    if band == 0:
                        nc.vector.tensor_copy(
                            out=q1b[kc][:, 0:1, :, :], in_=q1b[kc][:, 1:2, :, :])

            # in-transform ops for one (half, kc, j): 16 per band.
            # d0=even[tc] (plane1), d1=odd[tc] (plane0), d2=even[tc+1],
            # d3=odd[tc+1]: t0=d0-d2, t1=d1+d2, t2=d1-d2, t3=d1-d3.
            IT_OPS = [(0, (1, 0), (1, 1), ALU.subtract),
                      (1, (0, 0), (1, 1), ALU.add),
                      (2, (0, 0), (1, 1), ALU.subtract),
                      (3, (0, 0), (0, 1), ALU.subtract)]

            def intf_op(vh, h, kc, oi, eng=None):
                j, (pa, oa), (pb, ob_), op = IT_OPS[oi]
                rr = slice(16 * h, 16 * h + 18)
                (eng or nc.vector).tensor_tensor(
                    vh[:, kc, j, :, :],
                    q1b[kc][:, rr, pa, oa:oa + TC],
                    q1b[kc][:, rr, pb, ob_:ob_ + TC], op)

            def new_vh():
                return vpool.tile([128, 2, 4, 18, TC], F16, tag="vh", name="vh")

            GROUPS0 = [(j, 2) for j in range(1, RB + 1, 2)] + [(RB + 1, 1)]
            GROUPS = [(j, 2) for j in range(0, RB + 2, 2)]

            CH2 = [(0, 18), (18, 36)]

            # ---- band 0 prologue ---------------------------------------
            for j, nr in GROUPS0:
                conv1_group(xr_bufs[0], j, nr)
            emit_pads(0)
            vh_cur = new_vh()
            for kc in range(2):
                for oi in range(4):
                    # DVE here: the band-0 head has no other DVE work
                    intf_op(vh_cur, 0, kc, oi, eng=nc.vector)
            xr_bufs[1] = load_xband(RB, CH2)

            # ---- bands: 8 strip-pairs of 4 output rows each -------------
            for band in range(NBAND):
                ob = obuf.tile([128, NPAIR, 8], F32, tag="ob", name="ob")
                nb = band + 1
                for gp in range(NPAIR):
                    h = gp // 4
                    if gp == 4:
                        vh_cur = vh_pending
                    lr0 = 4 * gp - 16 * h
                    q2t = q2pool.tile([128, 2, 4, 2, TC], F16, tag="q2t",
                                      name="q2t")
                    for mc in range(2):
                        ps = [wps.tile([128, 4, TC], F32, tag="wps",
                                       name=f"wps{j}") for j in range(4)]
                        for j in range(4):
                            n = 0
                            for kc in range(2):
                                for dr in range(3):
                                    nc.tensor.matmul(
                                        ps[j], w2u_sb[j, dr, kc, mc],
                                        vh_cur[:, kc, j, lr0 + dr:lr0 + dr + 4, :],
                                        start=(n == 0), stop=(n == 5))
                                    n += 1
                        # out-transform: y0 = m0+m1+m2, y1 = m1-m2-m3
                        yt = ypool.tile([128, 4, 2, TC], F32, tag="yt", name="yt")
                        wa = wtp.tile([128, 4, TC], F32, tag="wa", name="wa")
                        wb = wtp.tile([128, 4, TC], F32, tag="wb", name="wb")
                        s1 = wtp.tile([128, 4, TC], F32, tag="s1", name="s1")
                        nc.scalar.activation(out=s1, in_=ps[1], func=AF.Copy)
                        nc.vector.tensor_tensor(wa, ps[0], s1, ALU.add)
                        nc.vector.tensor_tensor(yt[:, :, 0, :], wa, ps[2], ALU.add)
                        nc.vector.tensor_tensor(wb, s1, ps[2], ALU.subtract)
                        nc.vector.tensor_tensor(yt[:, :, 1, :], wb, ps[3],
                                                ALU.subtract)
                        nc.scalar.activation(out=q2t[:, mc], in_=yt,
                                             func=AF.Relu, bias=b2_sb[mc],
                                             scale=1.0)
                    # logits: [128 tc-px, K] per (r, s) chunk, q2 stationary
                    pl = psl.tile([128, 8, K], F32, tag="lps", name="lps")
                    for cc in range(8):
                        r, s = cc // 2, cc % 2
                        for kc in range(2):
                            nc.tensor.matmul(
                                pl[:, cc, :], q2t[:, kc, r, s, :],
                                muw_sb[kc], start=(kc == 0), stop=(kc == 1))
                    # softmax over K (free axis) + label contraction
                    li = smx.tile([128, 8, K], F32, tag="li", name="li")
                    nc.vector.tensor_tensor(
                        li, pl,
                        bp_sb.unsqueeze(1).to_broadcast([128, 8, K]), ALU.add)
                    mx = smx.tile([128, 8], F32, tag="mx", name="mx")
                    nc.vector.reduce_max(mx, li, axis=AX.X)
                    ls = smx.tile([128, 8, K], F32, tag="ls", name="ls")
                    nc.vector.tensor_tensor(
                        ls, li,
                        mx.unsqueeze(2).to_broadcast([128, 8, K]), ALU.subtract)
                    ex = smx.tile([128, 8, K], F32, tag="ex", name="ex")
                    nc.scalar.activation(out=ex, in_=ls, func=AF.Exp)
                    el = smx.tile([128, 8, K], F32, tag="el", name="el")
                    nc.vector.tensor_tensor(
                        el, ex,
                        lab_sb.unsqueeze(1).to_broadcast([128, 8, K]), ALU.mult)
                    ssum = smx.tile([128, 8], F32, tag="ssum", name="ssum")
                    nc.vector.reduce_sum(ssum, ex, axis=AX.X)
                    wsum = smx.tile([128, 8], F32, tag="wsum", name="wsum")
                    nc.vector.reduce_sum(wsum, el, axis=AX.X)
                    rs = smx.tile([128, 8], F32, tag="rs", name="rs")
                    nc.vector.reciprocal(rs, ssum)
                    nc.vector.tensor_tensor(ob[:, gp], wsum, rs, ALU.mult)

                    # ---- cross-band interleaves ------------------------
                    if gp < 2:
                        # second-half V of current band
                        if gp == 0:
                            vh_pending = new_vh()
                        for o in range(4 * gp, 4 * gp + 4):
                            intf_op(vh_pending, 1, o // 4, o % 4)
                    if nb < NBAND:
                        if gp in (1, 2) and nb >= 2:
                            if gp == 1:
                                xr_bufs[nb] = load_xband(RB * nb, CH2[:1])
                            else:
                                lo, hi = CH2[1]
                                nc.sync.dma_start(
                                    out=xr_bufs[nb][0:64, lo:hi, :],
                                    in_=xsrc(RB * nb, lo, hi, 0))
                                nc.sync.dma_start(
                                    out=xr_bufs[nb][64:128, lo:hi, :],
                                    in_=xsrc(RB * nb, lo, hi, W + 2))
                        if 2 <= gp <= 5:
                            i0 = (gp - 2) * 17 // 4
                            i1 = (gp - 1) * 17 // 4
                            for j, nr in GROUPS[i0:i1]:
                                conv1_group(xr_bufs[nb], j, nr)
                            if gp == 5:
                                emit_pads(nb)
                        if gp >= 6:
                            if gp == 6:
                                vh_nx = new_vh()
                            for o in range(4 * (gp - 6), 4 * (gp - 6) + 4):
                                intf_op(vh_nx, 0, o // 4, o % 4)

                nc.sync.dma_start(
                    out=outd.ap()[:, band * NPAIR * 8:(band + 1) * NPAIR * 8],
                    in_=ob)
                if nb < NBAND:
                    vh_cur = vh_nx

    nc.compile()
    return nc


def prep_inputs(x, w1, b1, w2, b2, w3, b3, mu, label):
    """Full inputs -> per-core in_maps."""
    w3m = w3[:, :, 0, 0]
    muW = 2.0 * (mu @ w3m)                                   # [K, Q]
    bpv = (2.0 * (mu @ b3) - (mu * mu).sum(1)).astype(np.float32)

    def pack_w(w1f, w2f):
        w1dp = np.empty((128, 6, 128), np.float32)
        for u in range(2):
            for dc in range(3):
                for mc in range(2):
                    w1dp[u * 64:(u + 1) * 64, dc * 2 + mc, :] = \
                        w1f[mc * 128:(mc + 1) * 128, :, u, dc].T
        w1ep = np.empty((128, 3, 128), np.float32)
        for dc in range(3):
            w1ep[0:64, dc, :] = w1f[0:128, :, 2, dc].T
            w1ep[64:128, dc, :] = w1f[128:256, :, 2, dc].T
        U = np.empty((4, 3, Q, Q), np.float32)
        for dr in range(3):
            g0, g1, g2 = (w2f[:, :, dr, dc] for dc in range(3))
            U[0, dr] = g0
            U[1, dr] = (g0 + g1 + g2) * 0.5
            U[2, dr] = (g1 - g0 - g2) * 0.5   # sign-folded for t2 = d1-d2
            U[3, dr] = g2
        w2up = np.empty((128, 48, 128), np.float32)
        for j in range(4):
            for dr in range(3):
                for kc in range(2):
                    for mc in range(2):
                        idx = ((j * 3 + dr) * 2 + kc) * 2 + mc
                        w2up[:, idx, :] = U[j, dr][
                            mc * 128:(mc + 1) * 128,
                            kc * 128:(kc + 1) * 128].T
        return (w1dp.astype(np.float16), w1ep.astype(np.float16),
                w2up.astype(np.float16))

    packs = {}
    packs[0] = pack_w(w1, w2)
    packs[1] = pack_w(w1[:, :, ::-1, :], w2[:, :, ::-1, :])

    muwp = np.empty((128, 2, K), np.float32)
    for kc in range(2):
        muwp[:, kc, :] = muW[:, 128 * kc:128 * (kc + 1)].T
    muwp = muwp.astype(np.float16)
    cstv = np.empty((128, 2 * K + 4), np.float32)
    cstv[:, 0:K] = bpv[None, :]
    cstv[:, K:2 * K] = label[None, :].astype(np.float32)
    for mc in range(2):
        cstv[:, 2 * K + mc] = b1[128 * mc:128 * (mc + 1)]
        cstv[:, 2 * K + 2 + mc] = b2[128 * mc:128 * (mc + 1)]

    rows = np.clip(np.arange(133) - 2, 0, H - 1)
    cols = np.clip(np.arange(W + 2) - 1, 0, W - 1)
    in_maps = []
    for core in range(NCORES):
        img, half = core // 2, core % 2
        xl = x[img] if half == 0 else x[img, :, ::-1, :]
        xhv = np.ascontiguousarray(xl[:, rows][:, :, cols]).astype(np.float16)
        w1dp, w1ep, w2up = packs[half]
        in_maps.append({
            'xh': xhv.reshape(-1), 'w1d': w1dp, 'w1e': w1ep,
            'w2u': w2up, 'muw': muwp, 'cst': cstv,
        })
    return in_maps


def gather(results, dtype=np.float32):
    out = np.empty((B, 1, H, W), dtype)
    for core in range(NCORES):
        img, half = core // 2, core % 2
        o = results[core]['out'].reshape(128, NBAND, NPAIR, 4, 2)
        # [p, band, gp, r, s]: row = 32*band + 4*gp + r, col = 2p + s
        o = o.transpose(1, 2, 3, 0, 4).reshape(128, W)
        if half == 0:
            out[img, 0, 0:128] = o
        else:
            out[img, 0, 128:256] = o[::-1]
    return out


def get_nc():
    if 'nc' not in _cached:
        _cached['nc'] = build_nc()
    return _cached['nc']


def kernel(x, w1, b1, w2, b2, w3, b3, mu, label, **run_kwargs):
    nc = get_nc()
    in_maps = prep_inputs(
        np.asarray(x, np.float32), np.asarray(w1, np.float32),
        np.asarray(b1, np.float32), np.asarray(w2, np.float32),
        np.asarray(b2, np.float32), np.asarray(w3, np.float32),
        np.asarray(b3, np.float32), np.asarray(mu, np.float32),
        np.asarray(label, np.float32))
    res = run_bass_kernel_spmd(nc, in_maps, core_ids=list(range(NCORES)),
                               **run_kwargs)
    out = gather(res.results)
    if run_kwargs:
        _cached['last_result'] = res
    return out
